# revision 1
# baseline (speedup 1.0000x reference)
"""CrossCompress unit kernel for Trainium2, 8-core data parallel.

Reference computation (per batch row b, D=128):
    item_out[b]   = v[b] * (e[b]@w_vv) + e[b] * (v[b]@w_ev) + bias_v
    entity_out[b] = v[b] * (e[b]@w_ve) + e[b] * (v[b]@w_ee) + bias_e

Strategy: pure data parallel over B=16384 rows -> 2048 rows/core.
Each core works in a transposed layout [D=128 partitions, batch free]:
the four per-row dot products become PE matmuls whose stationary operand
is the (D,1) weight replicated across 128 columns -- one matmul both
computes the dots AND broadcasts the result down all partitions. The
combine is then 4 tensor_tensor multiplies + 2 fused scalar_tensor_tensor
(bias add per-partition + sum) on DVE. fp32 throughout (fp32r measured at
~1e-3 error on HW - too lossy).

Walrus CoreV3 codegen accepts only ONE embedded sync wait per
instruction; the kernel is structured so every Matmult needs at most one
wait (consts ride the first chunk's DMA; the first matmul of each
super-tile targets a never-reused PSUM bank), and a post-pass splits any
remaining multi-wait instruction (e.g. the framework drain) into
single-wait NoOps.
"""
import sys
sys.path.insert(0, '/opt/trn_rl_repo')
import numpy as np
import bass_rust
import concourse.bass as bass
import concourse.tile as tile
from concourse import mybir
from concourse.bass_utils import run_bass_kernel_spmd
from concourse.tile_rust import add_dep_helper

B, D = 16384, 128
NCORES = 8
RPC = B // NCORES          # rows per core = 2048
N = 512                    # batch columns per super-tile
NST = RPC // N             # super-tiles per core = 4
CW = 4 * D + 2             # const block: 4 replicated weights + 2 biases

F32 = mybir.dt.float32


def _build():
    nc = bass.Bass("TRN2", target_bir_lowering=False, debug=False,
                   num_devices=NCORES)
    # flat input per core: [D, CW + NST*2*N]: [consts | st0 v | st0 e | ...]
    xin = nc.dram_tensor("xin", [D, CW + NST * 2 * N], F32,
                         kind="ExternalInput").ap()
    out = nc.dram_tensor("out", [D, NST, 2, N], F32,
                         kind="ExternalOutput").ap()

    with tile.TileContext(nc) as tc:
        with tc.tile_pool(name="c0", bufs=1) as c0_pool, \
             tc.tile_pool(name="io", bufs=NST) as io_pool, \
             tc.tile_pool(name="tmp", bufs=2) as tmp_pool, \
             tc.tile_pool(name="ps", bufs=2, space="PSUM") as ps_pool:

            # st=0 load: consts + first chunk in ONE dma
            c0_sb = c0_pool.tile([D, CW + 2 * N], F32)
            nc.sync.dma_start(out=c0_sb[:], in_=xin[:, 0:CW + 2 * N])
            w_sb = c0_sb[:, 0:4 * D]
            bv_sb = c0_sb[:, 4 * D:4 * D + 1]
            be_sb = c0_sb[:, 4 * D + 1:CW]

            for st in range(NST):
                if st == 0:
                    v_sb = c0_sb[:, CW:CW + N]
                    e_sb = c0_sb[:, CW + N:CW + 2 * N]
                else:
                    ve_sb = io_pool.tile([D, 2 * N], F32, tag="ve")
                    off = CW + st * 2 * N
                    nc.sync.dma_start(out=ve_sb[:], in_=xin[:, off:off + 2 * N])
                    v_sb = ve_sb[:, 0:N]
                    e_sb = ve_sb[:, N:2 * N]

                # 4 dot+broadcast matmuls into one double-buffered 4-bank tile
                s4 = ps_pool.tile([D, 4, N], F32, tag="s4")
                nc.tensor.matmul(s4[:, 0], w_sb[:, 0 * D:1 * D], e_sb,
                                 start=True, stop=True)
                nc.tensor.matmul(s4[:, 1], w_sb[:, 1 * D:2 * D], v_sb,
                                 start=True, stop=True)
                nc.tensor.matmul(s4[:, 2], w_sb[:, 2 * D:3 * D], e_sb,
                                 start=True, stop=True)
                nc.tensor.matmul(s4[:, 3], w_sb[:, 3 * D:4 * D], v_sb,
                                 start=True, stop=True)

                # products on DVE (PSUM-capable); bias+sum TSTs on GPSIMD
                o_sb = io_pool.tile([D, 2 * N], F32, tag="o")
                t1 = tmp_pool.tile([D, N], F32, tag="t1")
                nc.vector.tensor_mul(t1[:], v_sb, s4[:, 0])
                t2 = tmp_pool.tile([D, N], F32, tag="t2")
                nc.vector.tensor_mul(t2[:], e_sb, s4[:, 1])
                ts1 = tmp_pool.tile([D, N], F32, tag="ts1")
                nc.gpsimd.tensor_add(ts1[:], t1[:], t2[:])
                nc.scalar.activation(o_sb[:, 0:N], ts1[:],
                                     mybir.ActivationFunctionType.Identity,
                                     bias=bv_sb, scale=1.0)
                t3 = tmp_pool.tile([D, N], F32, tag="t3")
                nc.vector.tensor_mul(t3[:], v_sb, s4[:, 2])
                t4 = tmp_pool.tile([D, N], F32, tag="t4")
                nc.vector.tensor_mul(t4[:], e_sb, s4[:, 3])
                ts2 = tmp_pool.tile([D, N], F32, tag="ts2")
                nc.gpsimd.tensor_add(ts2[:], t3[:], t4[:])
                nc.scalar.activation(o_sb[:, N:2 * N], ts2[:],
                                     mybir.ActivationFunctionType.Identity,
                                     bias=be_sb, scale=1.0)

                nc.sync.dma_start(out=out[:, st], in_=o_sb[:])
    _split_multiwaits(nc)
    return nc


def _split_multiwaits(nc):
    """Split instructions carrying >1 sync wait into single-wait NoOps
    inserted just before them on the same engine queue."""
    n = 0
    for b in nc.m.functions[0].blocks:
        insts = b.instructions
        new = []
        for inst in insts:
            si = inst.sync_info
            if si is not None and si.on_wait and len(si.on_wait) > 1:
                waits = list(si.on_wait)
                for k, w in enumerate(waits[:-1]):
                    nop = mybir.InstNoOp(name=f"{inst.name}-sw{k}",
                                         ins=[], outs=[])
                    nop.engine = inst.engine
                    nop.sync_info = bass_rust.SyncInfo(on_wait=[w],
                                                       on_update=[])
                    nc.register_instruction(nop)
                    new.append(nop)
                    n += 1
                si.on_wait = [waits[-1]]
            new.append(inst)
        insts[:] = new
    return n


_NC = None


def _get_nc():
    global _NC
    if _NC is None:
        _NC = _build()
    return _NC


def _make_in_maps(v, e, w_vv, w_ve, w_ev, w_ee, bias_v, bias_e):
    cst = np.empty((D, CW), np.float32)
    cst[:, 0 * D:1 * D] = np.repeat(w_vv.reshape(D, 1), D, axis=1)
    cst[:, 1 * D:2 * D] = np.repeat(w_ev.reshape(D, 1), D, axis=1)
    cst[:, 2 * D:3 * D] = np.repeat(w_ve.reshape(D, 1), D, axis=1)
    cst[:, 3 * D:4 * D] = np.repeat(w_ee.reshape(D, 1), D, axis=1)
    cst[:, 4 * D] = bias_v.reshape(D)
    cst[:, 4 * D + 1] = bias_e.reshape(D)

    vT = np.ascontiguousarray(v.T)   # [D, B]
    eT = np.ascontiguousarray(e.T)
    in_maps = []
    for c in range(NCORES):
        xin = np.empty((D, CW + NST * 2 * N), np.float32)
        xin[:, 0:CW] = cst
        base = c * RPC
        for st in range(NST):
            off = CW + st * 2 * N
            lo = base + st * N
            xin[:, off:off + N] = vT[:, lo:lo + N]
            xin[:, off + N:off + 2 * N] = eT[:, lo:lo + N]
        in_maps.append({"xin": xin})
    return in_maps


def _run(in_maps, trace=False):
    return run_bass_kernel_spmd(_get_nc(), in_maps, list(range(NCORES)),
                                trace=trace)


def kernel(item_embedding, entity_embedding, w_vv, w_ve, w_ev, w_ee,
           bias_v, bias_e, _trace=False, _res_out=None):
    v = np.asarray(item_embedding, np.float32).reshape(B, D)
    e = np.asarray(entity_embedding, np.float32).reshape(B, D)
    in_maps = _make_in_maps(
        v, e,
        np.asarray(w_vv, np.float32), np.asarray(w_ve, np.float32),
        np.asarray(w_ev, np.float32), np.asarray(w_ee, np.float32),
        np.asarray(bias_v, np.float32), np.asarray(bias_e, np.float32))
    res = _run(in_maps, trace=_trace)
    if _res_out is not None:
        _res_out.append(res)
    item = np.empty((B, D, 1), np.float32)
    ent = np.empty((B, D, 1), np.float32)
    for c in range(NCORES):
        o = res.results[c]["out"]            # [D, NST, 2, N]
        item[c * RPC:(c + 1) * RPC, :, 0] = \
            o[:, :, 0, :].reshape(D, RPC).T
        ent[c * RPC:(c + 1) * RPC, :, 0] = \
            o[:, :, 1, :].reshape(D, RPC).T
    return (item, ent)



# revision 4
# speedup vs baseline: 1.2078x; 1.2078x over previous
"""CrossCompress unit kernel for Trainium2, 8-core data parallel.

Reference computation (per batch row b, D=128):
    item_out[b]   = v[b] * (e[b]@w_vv) + e[b] * (v[b]@w_ev) + bias_v
    entity_out[b] = v[b] * (e[b]@w_ve) + e[b] * (v[b]@w_ee) + bias_e

Strategy: pure data parallel over B=16384 rows -> 2048 rows/core.
Each core works in a transposed layout [D=128 partitions, batch free]:
the four per-row dot products become PE matmuls whose stationary operand
is the (D,1) weight replicated across 128 columns -- one matmul both
computes the dots AND broadcasts the result down all partitions. The
combine is then 4 tensor_tensor multiplies + 2 fused tensor adds +
2 per-partition bias activations spread over DVE / GPSIMD / Act.

All I/O and SBUF data is fp16 (PE runs fp16 at 1 cycle/row vs 4 for
fp32; DMA bytes halve). PSUM accumulation stays fp32. Measured global
relative error ~1e-3, comfortably inside the 2e-2 gate.

Walrus CoreV3 codegen accepts only ONE embedded sync wait per
instruction; the kernel is structured so every Matmult needs at most one
wait (consts ride the first chunk's DMA; the first matmul of each
super-tile targets a never-reused PSUM bank), and a post-pass splits any
remaining multi-wait instruction (e.g. the framework drain) into
single-wait NoOps.
"""
import sys
sys.path.insert(0, '/opt/trn_rl_repo')
import numpy as np
import bass_rust
import concourse.bass as bass
import concourse.tile as tile
from concourse import mybir
from concourse.bass_utils import run_bass_kernel_spmd
from concourse.tile_rust import add_dep_helper

B, D = 16384, 128
NCORES = 8
RPC = B // NCORES          # rows per core = 2048
N = 512                    # batch columns per super-tile
NST = RPC // N             # super-tiles per core = 4
CW = 4 * D + 2             # const block: 4 replicated weights + 2 biases

F32 = mybir.dt.float32
F16 = mybir.dt.float16


def _build():
    nc = bass.Bass("TRN2", target_bir_lowering=False, debug=False,
                   num_devices=NCORES)
    # flat input per core: [D, CW + NST*2*N]: [consts | st0 v | st0 e | ...]
    xin = nc.dram_tensor("xin", [D, CW + NST * 2 * N], F16,
                         kind="ExternalInput").ap()
    out = nc.dram_tensor("out", [D, NST, 2, N], F16,
                         kind="ExternalOutput").ap()

    with tile.TileContext(nc) as tc:
        with tc.tile_pool(name="c0", bufs=1) as c0_pool, \
             tc.tile_pool(name="io", bufs=NST) as io_pool, \
             tc.tile_pool(name="tmp", bufs=2) as tmp_pool, \
             tc.tile_pool(name="ps", bufs=2, space="PSUM") as ps_pool:

        # st=0 load: consts + first chunk in ONE dma
            c0_sb = c0_pool.tile([D, CW + 2 * N], F16)
            nc.sync.dma_start(out=c0_sb[:], in_=xin[:, 0:CW + 2 * N])
            w_sb = c0_sb[:, 0:4 * D]
            bv_sb = c0_sb[:, 4 * D:4 * D + 1]
            be_sb = c0_sb[:, 4 * D + 1:CW]

            for st in range(NST):
                if st == 0:
                    v_sb = c0_sb[:, CW:CW + N]
                    e_sb = c0_sb[:, CW + N:CW + 2 * N]
                else:
                    ve_sb = io_pool.tile([D, 2 * N], F16, tag="ve")
                    off = CW + st * 2 * N
                    nc.sync.dma_start(out=ve_sb[:], in_=xin[:, off:off + 2 * N])
                    v_sb = ve_sb[:, 0:N]
                    e_sb = ve_sb[:, N:2 * N]

                # 4 dot+broadcast matmuls into one double-buffered 4-bank tile
                s4 = ps_pool.tile([D, 4, N], F32, tag="s4")
                nc.tensor.matmul(s4[:, 0], w_sb[:, 0 * D:1 * D], e_sb,
                                 start=True, stop=True)
                nc.tensor.matmul(s4[:, 1], w_sb[:, 1 * D:2 * D], v_sb,
                                 start=True, stop=True)
                nc.tensor.matmul(s4[:, 2], w_sb[:, 2 * D:3 * D], e_sb,
                                 start=True, stop=True)
                nc.tensor.matmul(s4[:, 3], w_sb[:, 3 * D:4 * D], v_sb,
                                 start=True, stop=True)

                # products on DVE (PSUM-capable); bias+sum TSTs on GPSIMD
                o_sb = io_pool.tile([D, 2 * N], F16, tag="o")
                t1 = tmp_pool.tile([D, N], F16, tag="t1")
                nc.vector.tensor_mul(t1[:], v_sb, s4[:, 0])
                t2 = tmp_pool.tile([D, N], F16, tag="t2")
                nc.vector.tensor_mul(t2[:], e_sb, s4[:, 1])
                ts1 = tmp_pool.tile([D, N], F16, tag="ts1")
                nc.gpsimd.tensor_add(ts1[:], t1[:], t2[:])
                nc.scalar.activation(o_sb[:, 0:N], ts1[:],
                                     mybir.ActivationFunctionType.Identity,
                                     bias=bv_sb, scale=1.0)
                t3 = tmp_pool.tile([D, N], F16, tag="t3")
                nc.vector.tensor_mul(t3[:], v_sb, s4[:, 2])
                t4 = tmp_pool.tile([D, N], F16, tag="t4")
                nc.vector.tensor_mul(t4[:], e_sb, s4[:, 3])
                ts2 = tmp_pool.tile([D, N], F16, tag="ts2")
                nc.gpsimd.tensor_add(ts2[:], t3[:], t4[:])
                nc.scalar.activation(o_sb[:, N:2 * N], ts2[:],
                                     mybir.ActivationFunctionType.Identity,
                                     bias=be_sb, scale=1.0)

                nc.sync.dma_start(out=out[:, st], in_=o_sb[:])
    _split_multiwaits(nc)
    return nc


def _split_multiwaits(nc):
    """Split instructions carrying >1 sync wait into single-wait NoOps
    inserted just before them on the same engine queue."""
    n = 0
    for b in nc.m.functions[0].blocks:
        insts = b.instructions
        new = []
        for inst in insts:
            si = inst.sync_info
            if si is not None and si.on_wait and len(si.on_wait) > 1:
                waits = list(si.on_wait)
                for k, w in enumerate(waits[:-1]):
                    nop = mybir.InstNoOp(name=f"{inst.name}-sw{k}",
                                         ins=[], outs=[])
                    nop.engine = inst.engine
                    nop.sync_info = bass_rust.SyncInfo(on_wait=[w],
                                                       on_update=[])
                    nc.register_instruction(nop)
                    new.append(nop)
                    n += 1
                si.on_wait = [waits[-1]]
            new.append(inst)
        insts[:] = new
    return n


_NC = None


def _get_nc():
    global _NC
    if _NC is None:
        _NC = _build()
    return _NC


def _make_in_maps(v, e, w_vv, w_ve, w_ev, w_ee, bias_v, bias_e):
    cst = np.empty((D, CW), np.float16)
    cst[:, 0 * D:1 * D] = np.repeat(w_vv.reshape(D, 1), D, axis=1)
    cst[:, 1 * D:2 * D] = np.repeat(w_ev.reshape(D, 1), D, axis=1)
    cst[:, 2 * D:3 * D] = np.repeat(w_ve.reshape(D, 1), D, axis=1)
    cst[:, 3 * D:4 * D] = np.repeat(w_ee.reshape(D, 1), D, axis=1)
    cst[:, 4 * D] = bias_v.reshape(D)
    cst[:, 4 * D + 1] = bias_e.reshape(D)

    vT = np.ascontiguousarray(v.T).astype(np.float16)   # [D, B]
    eT = np.ascontiguousarray(e.T).astype(np.float16)
    in_maps = []
    for c in range(NCORES):
        xin = np.empty((D, CW + NST * 2 * N), np.float16)
        xin[:, 0:CW] = cst
        base = c * RPC
        for st in range(NST):
            off = CW + st * 2 * N
            lo = base + st * N
            xin[:, off:off + N] = vT[:, lo:lo + N]
            xin[:, off + N:off + 2 * N] = eT[:, lo:lo + N]
        in_maps.append({"xin": xin})
    return in_maps


def _run(in_maps, trace=False):
    return run_bass_kernel_spmd(_get_nc(), in_maps, list(range(NCORES)),
                                trace=trace)


def kernel(item_embedding, entity_embedding, w_vv, w_ve, w_ev, w_ee,
           bias_v, bias_e, _trace=False, _res_out=None):
    v = np.asarray(item_embedding, np.float32).reshape(B, D)
    e = np.asarray(entity_embedding, np.float32).reshape(B, D)
    in_maps = _make_in_maps(
        v, e,
        np.asarray(w_vv, np.float32), np.asarray(w_ve, np.float32),
        np.asarray(w_ev, np.float32), np.asarray(w_ee, np.float32),
        np.asarray(bias_v, np.float32), np.asarray(bias_e, np.float32))
    res = _run(in_maps, trace=_trace)
    if _res_out is not None:
        _res_out.append(res)
    item = np.empty((B, D, 1), np.float32)
    ent = np.empty((B, D, 1), np.float32)
    for c in range(NCORES):
        o = res.results[c]["out"]            # [D, NST, 2, N] fp16
        item[c * RPC:(c + 1) * RPC, :, 0] = \
            o[:, :, 0, :].reshape(D, RPC).T
        ent[c * RPC:(c + 1) * RPC, :, 0] = \
            o[:, :, 1, :].reshape(D, RPC).T
    return (item, ent)
